# revision 1
# baseline (speedup 1.0000x reference)
"""MHGCN kernel for 8 Trainium2 NeuronCores.

Strategy: row-shard the [7,4096,4096] A_stack across 8 cores (512 rows each,
read once ~58.7MB/core = the memory floor).  Each core builds the transposed
row-block of final_A:
    FT[k, m] = merged[m, k] + merged[k, m] + s*tanh_interaction[m, k]
where m runs over the core's 512 rows and k over all 4096 columns.
 - merged[m, k]   : local weighted sum over 7 relations, PE-transposed.
 - merged[k, m]   : column block of merged, obtained via AllToAll of the
                    512-column chunks of every core's row block (8MB).
 - interaction    : tanh(sum_i (1.5 R_i + sign R_i) * sum_{j!=i} 0.4 M_ij R_j),
                    local, PE-transposed, scaled by interaction_strength.
Both GCN layers are then plain FT^T @ Y matmuls; U1 is AllGathered (1MB)
between layers.  struct_adj = (encode*sw) @ encode^T is rank-7 and computed
locally without materializing it.
"""
import sys

sys.path.insert(0, "/opt/trn_rl_repo")

import numpy as np

import bass_rust
import concourse.bass as bass
import concourse.tile as tile
from concourse import mybir
from concourse.bass_utils import run_bass_kernel_spmd
from concourse.masks import make_identity
from concourse.vector_clock import ScopedClock

F32 = mybir.dt.float32
BF16 = mybir.dt.bfloat16
AF = mybir.ActivationFunctionType
OP = mybir.AluOpType

P = 128
N = 4096
NFEAT = 128
OUT = 64
NREL = 7
NCORES = 8
ROWS = N // NCORES        # 512 rows per core
NT = ROWS // P            # 4 row tiles per core
KT = N // P               # 32 k tiles
C = 1024                  # streaming column chunk
NCH = N // C              # 4 chunks per row tile
DST = ROWS                # alltoall chunk width (512)


def _patched_drain_and_barrier(self, tick_clock, wait_clock):
    # Stock Tile attaches every outstanding proc's sem wait to one Drain;
    # this walrus build caps sync waits per instruction, so split them
    # into single-wait drains.
    drain_inst = self.nc.sync.drain()
    wait_clock.add_sem_waits(
        drain_inst.ins, ScopedClock({None: tick_clock.global_clock})
    )
    si = drain_inst.ins.sync_info
    if si is not None and len(si.on_wait) > 1:
        waits = list(si.on_wait)
        si.on_wait = [waits[0]]
        for w in waits[1:]:
            extra = self.nc.sync.drain()
            extra.ins.sync_info = bass_rust.SyncInfo(on_wait=[w], on_update=[])
    self.nc.all_engine_barrier()
    assert self.sems is not None
    popped = self.nc._tile_sem_poison_stack.pop()
    assert popped is self._sem_poison
    self.nc.clear_and_free_semaphores(list(self.sems.allocated().values()))
    self.nc.all_engine_barrier()


tile.TileContext._drain_and_barrier = _patched_drain_and_barrier


def _split_multi_waits(nc, limit=1):
    """Walrus in this container caps sync-wait commands per instruction.
    Hoist all-but-`limit` waits of any instruction onto single-wait NoOps
    inserted just before it on the same engine queue."""
    cnt = 0
    for fn in nc.m.functions:
        for blk in fn.blocks:
            lst = list(blk.instructions)
            out = []
            changed = False
            for inst in lst:
                si = inst.sync_info
                if si is not None and len(si.on_wait) > limit:
                    waits = list(si.on_wait)
                    for w in waits[:-limit]:
                        n = bass_rust.InstNoOp(name=f"wsplit-{cnt}")
                        cnt += 1
                        n.engine = inst.engine
                        n.bass_nofuse = True
                        n.sync_info = bass_rust.SyncInfo(on_wait=[w],
                                                         on_update=[])
                        nc.register_instruction(n, overwrite=True)
                        out.append(n)
                    si.on_wait = waits[-limit:]
                    changed = True
                out.append(inst)
            if changed:
                blk.instructions = out
    return cnt


def _normalize(nc, pool, psum, x, out_dram, i):
    """l2-normalize rows of x [P, OUT] and DMA to out_dram[i*P:(i+1)*P]."""
    sq = pool.tile([P, OUT], F32, tag="nrm_sq")
    nrm = pool.tile([P, 1], F32, tag="nrm_n")
    nc.vector.tensor_tensor(sq[:], x[:], x[:], OP.mult)
    nc.vector.tensor_reduce(nrm[:], sq[:], mybir.AxisListType.X, OP.add)
    nr = pool.tile([P, 1], F32, tag="nrm_r")
    nc.scalar.activation(nr[:], nrm[:], AF.Sqrt)
    nc.vector.tensor_scalar(nr[:], nr[:], 1e-12, None, OP.max)
    ninv = pool.tile([P, 1], F32, tag="nrm_i")
    nc.vector.reciprocal(ninv[:], nr[:])
    y = pool.tile([P, OUT], F32, tag="nrm_y")
    nc.vector.tensor_scalar(y[:], x[:], ninv[:], None, OP.mult)
    nc.sync.dma_start(out=out_dram[i * P:(i + 1) * P, :], in_=y[:])


def build_nc():
    nc = bass.Bass()

    a_strip = nc.dram_tensor("a_strip", [NREL, ROWS, N], BF16, kind="ExternalInput")
    featT = nc.dram_tensor("featT", [NFEAT, N], F32, kind="ExternalInput")
    encode = nc.dram_tensor("encode", [N, NREL], F32, kind="ExternalInput")
    enc_rows = nc.dram_tensor("enc_rows", [ROWS, NREL], F32, kind="ExternalInput")
    W1 = nc.dram_tensor("W1", [NFEAT, OUT], F32, kind="ExternalInput")
    W2 = nc.dram_tensor("W2", [OUT, OUT], F32, kind="ExternalInput")
    b1 = nc.dram_tensor("b1", [1, OUT], F32, kind="ExternalInput")
    b2 = nc.dram_tensor("b2", [1, OUT], F32, kind="ExternalInput")
    wb = nc.dram_tensor("wb", [1, NREL], F32, kind="ExternalInput")
    ri = nc.dram_tensor("ri", [1, 9], F32, kind="ExternalInput")
    s_ = nc.dram_tensor("s_", [1, 1], F32, kind="ExternalInput")
    sw = nc.dram_tensor("sw", [NREL, 1], F32, kind="ExternalInput")

    o_res = nc.dram_tensor("o_res", [ROWS, OUT], F32, kind="ExternalOutput")
    o_b1 = nc.dram_tensor("o_b1", [ROWS, OUT], F32, kind="ExternalOutput")
    o_b2 = nc.dram_tensor("o_b2", [ROWS, OUT], F32, kind="ExternalOutput")

    groups = [list(range(NCORES))]

    with tile.TileContext(nc) as tc:
        with (
            tc.tile_pool(name="persist", bufs=1) as pp,
            tc.tile_pool(name="dram", bufs=1, space="DRAM") as dpool,
        ):
            # ---- constants / small tensors ----
            ident = pp.tile([P, P], F32)
            make_identity(nc, ident)
            identb = pp.tile([P, P], BF16)
            nc.vector.tensor_copy(identb[:], ident[:])

            ones_1p = pp.tile([1, P], F32)
            nc.vector.memset(ones_1p[:], 1.0)

            # scalar staging: [0:7]=w_r, [7:16]=M flat, [16]=s
            sstage = pp.tile([1, 17], F32)
            nc.sync.dma_start(out=sstage[:, 0:NREL], in_=wb[:])
            nc.sync.dma_start(out=sstage[:, NREL:NREL + 9], in_=ri[:])
            nc.sync.dma_start(out=sstage[:, 16:17], in_=s_[:])

            W1t = pp.tile([NFEAT, OUT], F32)
            nc.sync.dma_start(out=W1t[:], in_=W1[:])
            W2t = pp.tile([OUT, OUT], F32)
            nc.sync.dma_start(out=W2t[:], in_=W2[:])
            b1st = pp.tile([1, OUT], F32)
            nc.sync.dma_start(out=b1st[:], in_=b1[:])
            b2st = pp.tile([1, OUT], F32)
            nc.sync.dma_start(out=b2st[:], in_=b2[:])
            swt = pp.tile([NREL, 1], F32)
            nc.sync.dma_start(out=swt[:], in_=sw[:])

            scal = pp.tile([P, 17], F32)
            b1b = pp.tile([P, OUT], F32)
            b2b = pp.tile([P, OUT], F32)
            with tc.tile_pool(name="ppsum", bufs=1, space="PSUM") as pps:
                pb = pps.tile([P, 17], F32, tag="pb")
                nc.tensor.matmul(pb[:], lhsT=ones_1p[:], rhs=sstage[:],
                                 start=True, stop=True)
                nc.vector.tensor_copy(scal[:], pb[:])
                pb1 = pps.tile([P, OUT], F32, tag="pb1")
                nc.tensor.matmul(pb1[:], lhsT=ones_1p[:], rhs=b1st[:],
                                 start=True, stop=True)
                nc.vector.tensor_copy(b1b[:], pb1[:])
                pb2 = pps.tile([P, OUT], F32, tag="pb2")
                nc.tensor.matmul(pb2[:], lhsT=ones_1p[:], rhs=b2st[:],
                                 start=True, stop=True)
                nc.vector.tensor_copy(b2b[:], pb2[:])

            # fp32 broadcast scalars (TensorScalarPtr requires fp32 scalars)
            scal04 = pp.tile([P, 9], F32)
            nc.vector.tensor_scalar(scal04[:], scal[:, NREL:NREL + 9], 0.4,
                                    None, OP.mult)

            def w_ap(r):
                return scal[:, r:r + 1]

            s_ap = scal[:, 16:17]

            def c04_ap(i, j):
                return scal04[:, 3 * i + j:3 * i + j + 1]

            # ---- persistent big tensors (bf16; PSUM accumulates fp32) ----
            FT = pp.tile([P, KT * ROWS], BF16)    # final_A^T: 32 k-tiles x [128, 512]
            YG = pp.tile([P, KT * 2 * OUT], BF16)  # [Y1 | G] per k-tile

            # ---- DRAM bounce buffers ----
            sendbuf = dpool.tile([N, DST], BF16)
            recvbuf = dpool.tile([N, DST], BF16)
            agin = dpool.tile([ROWS, OUT], F32)
            agout = dpool.tile([N, OUT], F32, addr_space="Shared")

            # ---- prep: Y1 = feature @ W1 ----
            with (
                tc.tile_pool(name="prep", bufs=1) as prep,
                tc.tile_pool(name="preppsum", bufs=2, space="PSUM") as prps,
            ):
                ftile = prep.tile([NFEAT, N], F32)
                nc.sync.dma_start(out=ftile[:], in_=featT[:])
                fbf = prep.tile([NFEAT, N], BF16)
                nc.vector.tensor_copy(fbf[:], ftile[:])
                W1b = pp.tile([NFEAT, OUT], BF16)
                nc.vector.tensor_copy(W1b[:], W1t[:])
                W2b = pp.tile([OUT, OUT], BF16)
                nc.vector.tensor_copy(W2b[:], W2t[:])
                # W12 = W1 @ W2 (via W1^T transpose), h = b1 @ W2
                pw1t = prps.tile([P, P], BF16, tag="prsm")
                nc.tensor.transpose(pw1t[:OUT, :NFEAT], W1b[:], identb[:])
                W1T = prep.tile([OUT, NFEAT], BF16)
                nc.vector.tensor_copy(W1T[:], pw1t[:OUT, :NFEAT])
                pw12 = prps.tile([NFEAT, OUT], F32, tag="prsm")
                nc.tensor.matmul(pw12[:], lhsT=W1T[:], rhs=W2b[:],
                                 start=True, stop=True)
                W12b = pp.tile([NFEAT, OUT], BF16)
                nc.vector.tensor_copy(W12b[:], pw12[:])
                b1v = prep.tile([OUT, 1], BF16)
                pb1t = prps.tile([OUT, 1], BF16, tag="prsm")
                b1bf = prep.tile([1, OUT], BF16)
                nc.vector.tensor_copy(b1bf[:], b1st[:])
                nc.tensor.transpose(pb1t[:], b1bf[:], identb[:1, :1])
                nc.vector.tensor_copy(b1v[:], pb1t[:])
                phh = prps.tile([1, OUT], F32, tag="prsm")
                nc.tensor.matmul(phh[:], lhsT=b1v[:], rhs=W2b[:],
                                 start=True, stop=True)
                hst = prep.tile([1, OUT], F32)
                nc.vector.tensor_copy(hst[:], phh[:])
                phb = prps.tile([P, OUT], F32, tag="prsm")
                nc.tensor.matmul(phb[:], lhsT=ones_1p[:], rhs=hst[:],
                                 start=True, stop=True)
                hb = pp.tile([P, OUT], F32)
                nc.vector.tensor_copy(hb[:], phb[:])

                for kt in range(KT):
                    pm = prps.tile([P, OUT], F32, tag="y1p")
                    nc.tensor.matmul(pm[:], lhsT=fbf[:, kt * P:(kt + 1) * P],
                                     rhs=W1b[:], start=True, stop=True)
                    nc.vector.tensor_copy(
                        YG[:, kt * 2 * OUT:kt * 2 * OUT + OUT], pm[:])
                    pg_ = prps.tile([P, OUT], F32, tag="gp")
                    nc.tensor.matmul(pg_[:], lhsT=fbf[:, kt * P:(kt + 1) * P],
                                     rhs=W12b[:], start=True, stop=True)
                    nc.vector.tensor_copy(
                        YG[:, kt * 2 * OUT + OUT:(kt + 1) * 2 * OUT], pg_[:])

            # ---- phase 1: stream A row block ----
            with (
                tc.tile_pool(name="rstr", bufs=2) as prr,
                tc.tile_pool(name="qstr", bufs=2) as pq,
                tc.tile_pool(name="estr", bufs=2) as pe,
                tc.tile_pool(name="tstr", bufs=2) as ptn,
                tc.tile_pool(name="mstr", bufs=2) as pm_,
                tc.tile_pool(name="astr", bufs=2) as pa,
                tc.tile_pool(name="strpsum", bufs=4, space="PSUM") as sps,
            ):
                for i in range(NT):
                    for q in range(NCH):
                        c0 = q * C
                        rb = []
                        for j in range(3):
                            rj = prr.tile([P, C], BF16, tag=f"r{j}")
                            nc.sync.dma_start(
                                out=rj[:],
                                in_=a_strip[j, i * P:(i + 1) * P, c0:c0 + C])
                            rb.append(rj)
                        # Q_j = 1.5*R_j + sign(R_j)
                        sg = []
                        for j in range(3):
                            sj = pq.tile([P, C], BF16, tag=f"s{j}")
                            nc.scalar.sign(sj[:], rb[j][:])
                            tq = pq.tile([P, C], BF16, tag="tmp", bufs=4)
                            nc.scalar.activation(tq[:], rb[j][:], AF.Copy,
                                                 scale=1.5)
                            nc.vector.tensor_tensor(sj[:], sj[:], tq[:], OP.add)
                            sg.append(sj)
                        # E_j = sum_{o!=j} 0.4*M_jo*R_o
                        ee = []
                        for j in range(3):
                            o1, o2 = [x for x in range(3) if x != j]
                            ej = pe.tile([P, C], BF16, tag=f"e{j}")
                            nc.scalar.activation(ej[:], rb[o1][:], AF.Copy,
                                                 scale=c04_ap(j, o1))
                            te = pe.tile([P, C], BF16, tag="tmp2", bufs=4)
                            nc.vector.tensor_scalar(te[:], rb[o2][:],
                                                    c04_ap(j, o2), None, OP.mult)
                            nc.vector.tensor_tensor(ej[:], ej[:], te[:], OP.add)
                            ee.append(ej)
                        # arg = sum_j Q_j*E_j ; tanh
                        for j in range(3):
                            nc.vector.tensor_tensor(ee[j][:], sg[j][:], ee[j][:],
                                                    OP.mult)
                        nc.vector.tensor_tensor(ee[0][:], ee[0][:], ee[1][:],
                                                OP.add)
                        nc.vector.tensor_tensor(ee[0][:], ee[0][:], ee[2][:],
                                                OP.add)
                        tT = ptn.tile([P, C], BF16, tag="tT")
                        nc.scalar.activation(tT[:], ee[0][:], AF.Tanh)

                        # merged row chunk: r0-3 on DVE, r4-6 scaled on ACT
                        mrow = pm_.tile([P, C], BF16, tag="mrow")
                        nc.vector.tensor_scalar(mrow[:], rb[0][:], w_ap(0), None,
                                                OP.mult)
                        for rel in (1, 2):
                            tm = pm_.tile([P, C], BF16, tag="tm", bufs=4)
                            nc.vector.tensor_scalar(tm[:], rb[rel][:], w_ap(rel),
                                                    None, OP.mult)
                            nc.vector.tensor_tensor(mrow[:], mrow[:], tm[:],
                                                    OP.add)
                        for rel in range(3, NREL):
                            ra = pa.tile([P, C], BF16, tag="ra")
                            nc.sync.dma_start(
                                out=ra[:],
                                in_=a_strip[rel, i * P:(i + 1) * P, c0:c0 + C])
                            tm = pm_.tile([P, C], BF16, tag="tm", bufs=4)
                            if rel >= 4:
                                nc.scalar.activation(tm[:], ra[:], AF.Copy,
                                                     scale=w_ap(rel))
                            else:
                                nc.vector.tensor_scalar(tm[:], ra[:], w_ap(rel),
                                                        None, OP.mult)
                            nc.vector.tensor_tensor(mrow[:], mrow[:], tm[:],
                                                    OP.add)

                        # send merged chunks to alltoall buffer (bf16)
                        for d in range(c0 // DST, (c0 + C) // DST):
                            nc.sync.dma_start(
                                out=sendbuf[d * DST + i * P:
                                            d * DST + (i + 1) * P, :],
                                in_=mrow[:, d * DST - c0:d * DST - c0 + DST])

                        # L = merged + s*tanh, transpose into FT (one pass)
                        lt = ptn.tile([P, C], BF16, tag="lt")
                        nc.vector.tensor_scalar(lt[:], tT[:], s_ap, None, OP.mult)
                        nc.vector.tensor_tensor(lt[:], lt[:], mrow[:], OP.add)
                        for t in range(C // P):
                            kt = c0 // P + t
                            fsl = FT[:, kt * ROWS + i * P:kt * ROWS + (i + 1) * P]
                            pt1 = sps.tile([P, P], BF16, tag="pt1")
                            nc.tensor.transpose(pt1[:], lt[:, t * P:(t + 1) * P],
                                                identb[:])
                            if t % 2:
                                nc.scalar.activation(fsl, pt1[:], AF.Copy)
                            else:
                                nc.vector.tensor_copy(fsl, pt1[:])

            # ---- phase 2: alltoall + add received column blocks ----
            nc.gpsimd.collective_compute(
                "AllToAll", OP.bypass, replica_groups=groups,
                ins=[sendbuf[:].opt()], outs=[recvbuf[:].opt()])

            with (
                tc.tile_pool(name="post", bufs=1) as post,
                tc.tile_pool(name="rcv", bufs=4) as prc,
                tc.tile_pool(name="postpsum", bufs=2, space="PSUM") as pops,
            ):
                # ---- struct branch (rank-7) ----
                encsb = post.tile([P, KT * NREL], F32)
                for kt in range(KT):
                    nc.sync.dma_start(out=encsb[:, kt * NREL:(kt + 1) * NREL],
                                      in_=encode[kt * P:(kt + 1) * P, :])
                encb = post.tile([P, KT * NREL], BF16)
                nc.vector.tensor_copy(encb[:], encsb[:])
                encT = post.tile([NREL, N], BF16)
                for kt in range(KT):
                    pte = pops.tile([P, P], BF16, tag="pp_tr")
                    nc.tensor.transpose(pte[:NREL, :],
                                        encb[:, kt * NREL:(kt + 1) * NREL],
                                        identb[:])
                    nc.scalar.activation(encT[:, kt * P:(kt + 1) * P],
                                         pte[:NREL, :], AF.Copy)
                encRsb = post.tile([P, NT * NREL], F32)
                for i in range(NT):
                    nc.sync.dma_start(out=encRsb[:, i * NREL:(i + 1) * NREL],
                                      in_=enc_rows[i * P:(i + 1) * P, :])
                encRb = post.tile([P, NT * NREL], BF16)
                nc.vector.tensor_copy(encRb[:], encRsb[:])
                encRT = post.tile([NREL, ROWS], BF16)
                for i in range(NT):
                    pte = pops.tile([P, P], BF16, tag="pp_tr")
                    nc.tensor.transpose(pte[:NREL, :],
                                        encRb[:, i * NREL:(i + 1) * NREL],
                                        identb[:])
                    nc.scalar.activation(encRT[:, i * P:(i + 1) * P],
                                         pte[:NREL, :], AF.Copy)

                # H1 = encode^T @ Y1, scaled by sw
                ph = pops.tile([NREL, OUT], F32, tag="pp_mm")
                for kt in range(KT):
                    nc.tensor.matmul(ph[:],
                                     lhsT=encb[:, kt * NREL:(kt + 1) * NREL],
                                     rhs=YG[:, kt * 2 * OUT:kt * 2 * OUT + OUT],
                                     start=(kt == 0), stop=(kt == KT - 1))
                H1p = post.tile([NREL, OUT], BF16)
                nc.scalar.activation(H1p[:], ph[:], AF.Copy, scale=swt[:])

                # U3 = encode @ H1p + b1 (full, replicated)
                U3sb = post.tile([P, KT * OUT], BF16)
                for kt in range(KT):
                    pm3 = pops.tile([P, OUT], F32, tag="pp_mm")
                    nc.tensor.matmul(pm3[:], lhsT=encT[:, kt * P:(kt + 1) * P],
                                     rhs=H1p[:], start=True, stop=True)
                    nc.vector.tensor_tensor(U3sb[:, kt * OUT:(kt + 1) * OUT],
                                            pm3[:], b1b[:], OP.add)

                # G2 = (encode^T @ U3) @ W2, scaled by sw
                pg = pops.tile([NREL, OUT], F32, tag="pp_mm")
                for kt in range(KT):
                    nc.tensor.matmul(pg[:],
                                     lhsT=encb[:, kt * NREL:(kt + 1) * NREL],
                                     rhs=U3sb[:, kt * OUT:(kt + 1) * OUT],
                                     start=(kt == 0), stop=(kt == KT - 1))
                Gsb = post.tile([NREL, OUT], BF16)
                nc.vector.tensor_copy(Gsb[:], pg[:])
                pgt = pops.tile([OUT, NREL], BF16, tag="pp_tr")
                nc.tensor.transpose(pgt[:], Gsb[:], identb[:NREL, :NREL])
                GT = post.tile([OUT, NREL], BF16)
                nc.vector.tensor_copy(GT[:], pgt[:])
                pg2 = pops.tile([NREL, OUT], F32, tag="pp_mm")
                nc.tensor.matmul(pg2[:], lhsT=GT[:], rhs=W2b[:],
                                 start=True, stop=True)
                G2p = post.tile([NREL, OUT], BF16)
                nc.scalar.activation(G2p[:], pg2[:], AF.Copy, scale=swt[:])

                # U4 rows = enc_rows @ G2p + b2
                U4sb = post.tile([P, NT * OUT], F32)
                for i in range(NT):
                    pm4 = pops.tile([P, OUT], F32, tag="pp_mm")
                    nc.tensor.matmul(pm4[:], lhsT=encRT[:, i * P:(i + 1) * P],
                                     rhs=G2p[:], start=True, stop=True)
                    nc.vector.tensor_tensor(U4sb[:, i * OUT:(i + 1) * OUT],
                                            pm4[:], b2b[:], OP.add)

                for kt in range(KT):
                    rt = prc.tile([P, DST], BF16, tag="rt")
                    nc.sync.dma_start(out=rt[:],
                                      in_=recvbuf[kt * P:(kt + 1) * P, :])
                    fsl = FT[:, kt * ROWS:(kt + 1) * ROWS]
                    nc.vector.tensor_tensor(fsl, fsl, rt[:], OP.add)

                # ---- fused [U1 | V] = final_A @ [Y1 | G]; allgather V early ----
                U1sb = post.tile([P, NT * OUT], F32)
                for i in range(NT):
                    pmv = pops.tile([P, 2 * OUT], F32, tag="pp_mm")
                    for kt in range(KT):
                        nc.tensor.matmul(
                            pmv[:],
                            lhsT=FT[:, kt * ROWS + i * P:kt * ROWS + (i + 1) * P],
                            rhs=YG[:, kt * 2 * OUT:(kt + 1) * 2 * OUT],
                            start=(kt == 0), stop=(kt == KT - 1))
                    vt = post.tile([P, OUT], F32, tag="vt", bufs=2)
                    nc.vector.tensor_copy(vt[:], pmv[:, OUT:2 * OUT])
                    nc.sync.dma_start(out=agin[i * P:(i + 1) * P, :], in_=vt[:])
                    nc.vector.tensor_tensor(U1sb[:, i * OUT:(i + 1) * OUT],
                                            pmv[:, 0:OUT], b1b[:], OP.add)
                nc.gpsimd.collective_compute(
                    "AllGather", OP.bypass, replica_groups=groups,
                    ins=[agin[:].opt()], outs=[agout[:].opt()])

                # ---- Y2' = V_full + b1@W2, cast bf16 ----
                Y2 = post.tile([P, KT * OUT], BF16)
                for kt in range(KT):
                    vtk = post.tile([P, OUT], F32, tag="vtk", bufs=4)
                    nc.sync.dma_start(out=vtk[:],
                                      in_=agout[kt * P:(kt + 1) * P, :])
                    nc.vector.tensor_tensor(Y2[:, kt * OUT:(kt + 1) * OUT],
                                            vtk[:], hb[:], OP.add)

                # ---- layer 2: U2 = final_A @ Y2 + b2 ----
                U2sb = post.tile([P, NT * OUT], F32)
                for i in range(NT):
                    pm = pops.tile([P, OUT], F32, tag="pp_mm")
                    for kt in range(KT):
                        nc.tensor.matmul(
                            pm[:],
                            lhsT=FT[:, kt * ROWS + i * P:kt * ROWS + (i + 1) * P],
                            rhs=Y2[:, kt * OUT:(kt + 1) * OUT],
                            start=(kt == 0), stop=(kt == KT - 1))
                    nc.vector.tensor_tensor(U2sb[:, i * OUT:(i + 1) * OUT],
                                            pm[:], b2b[:], OP.add)

                # ---- combine + normalize + store ----
                for i in range(NT):
                    sl = slice(i * OUT, (i + 1) * OUT)
                    br1 = post.tile([P, OUT], F32, tag="br1", bufs=2)
                    nc.vector.tensor_tensor(br1[:], U1sb[:, sl], U2sb[:, sl],
                                            OP.add)
                    nc.vector.tensor_scalar(br1[:], br1[:], 0.5, None, OP.mult)
                    res = post.tile([P, OUT], F32, tag="res", bufs=2)
                    nc.vector.tensor_tensor(res[:], br1[:], U4sb[:, sl], OP.add)
                    nc.vector.tensor_scalar(res[:], res[:], 0.5, None, OP.mult)
                    _normalize(nc, post, pops, res, o_res, i)
                    _normalize(nc, post, pops, br1, o_b1, i)
                    u4 = post.tile([P, OUT], F32, tag="u4n", bufs=2)
                    nc.vector.tensor_copy(u4[:], U4sb[:, sl])
                    _normalize(nc, post, pops, u4, o_b2, i)

    _split_multi_waits(nc)
    return nc


_NC_CACHE = None


def get_nc():
    global _NC_CACHE
    if _NC_CACHE is None:
        _NC_CACHE = build_nc()
    return _NC_CACHE


def make_in_maps(feature, A_stack, encode, W1, b1, W2, b2, weight_b,
                 relation_interaction, interaction_strength, struct_weight):
    f32 = lambda x: np.ascontiguousarray(np.asarray(x, dtype=np.float32))
    featT = f32(np.asarray(feature, np.float32).T)
    enc = f32(encode)
    common = dict(
        featT=featT,
        encode=enc,
        W1=f32(W1),
        W2=f32(W2),
        b1=f32(np.reshape(b1, (1, OUT))),
        b2=f32(np.reshape(b2, (1, OUT))),
        wb=f32(np.reshape(np.asarray(weight_b, np.float32)[:, 0], (1, NREL))),
        ri=f32(np.reshape(relation_interaction, (1, 9))),
        s_=f32(np.reshape(interaction_strength, (1, 1))),
        sw=f32(np.reshape(struct_weight, (NREL, 1))),
    )
    in_maps = []
    import ml_dtypes
    A = np.asarray(A_stack, np.float32).astype(ml_dtypes.bfloat16)
    for c in range(NCORES):
        rows = slice(c * ROWS, (c + 1) * ROWS)
        m = dict(common)
        m["a_strip"] = np.ascontiguousarray(A[:, rows, :])
        m["enc_rows"] = f32(enc[rows])
        in_maps.append(m)
    return in_maps


def run(inputs, trace=False, tmpdir=None):
    nc = get_nc()
    in_maps = make_in_maps(**inputs)
    kres = run_bass_kernel_spmd(nc, in_maps, list(range(NCORES)),
                                trace=trace, tmpdir=tmpdir)
    res = kres.results
    result = np.concatenate([res[c]["o_res"] for c in range(NCORES)], axis=0)
    branch1 = np.concatenate([res[c]["o_b1"] for c in range(NCORES)], axis=0)
    branch2 = np.concatenate([res[c]["o_b2"] for c in range(NCORES)], axis=0)
    return (result, branch1, branch2), kres


def kernel(**inputs):
    return run(inputs)[0]

